# revision 6
# baseline (speedup 1.0000x reference)
"""Multi-head attention (B=2, S=2048, D=1024, H=16) on 8 trn2 NeuronCores.

v5: swapped-ctx redesign.  Measured facts this build is shaped around
(microbench on this backend):
  - matmul wall time ~= 0.516ns x moving-cols (+~4ns), independent of
    dtype, contraction depth, and stationary reload (ldweights is free).
  - two 64-contraction-row matmuls at disjoint row quadrants run fully
    concurrently IF they target different PSUM banks.
  - matmul start=True zeroes the WHOLE psum bank; tiles are allocated
    bank-granular, so start=True is safe per-tile, but interleaved
    accumulation groups inside one tile use DVE pre-zero + start=False.
  - ACT activation ~757ns per [128,512] tile; DVE tensor_scalar
    (Schraudolph exp) ~331ns; DVE copy ~466ns.
Design:
  - scores as baseline: quadrant-paired 64-contraction matmuls -> [k,q].
  - ctx SWAPPED: stationary = 128x128 pt block, moving = [V|1] (65
    cols) -> ct[q, h, hd|den] in PSUM; 8x65-col matmuls per kt (301ns)
    instead of 2x512-col (530ns).
  - normalization via per-partition denominator: reciprocal with
    free-dim-broadcast input + one mul -> no PE broadcast matmuls.
  - ctn transposed back to [d, q] with one PE transpose per 128-q block
    (identity built on gpsimd), staged to SBUF f32r by one DVE copy.
  - exp split 16 ACT / 16 DVE-Schraudolph per 32 half-tiles, strict
    XD/DX alternation (no double-ACT kts); PSUM reads throttle both
    engines to ~725ns/tile so the pair runs in lockstep.
  - ALL matmul operands bf16 (qt/kt/ctn/wot included): the PE costs
    ~300ns per f32r<->bf16 dtype switch, which dominated earlier
    revisions (~2 switches/kt ~= 77us).
  - ctx deferral depth 6 + 20-deep pt ring give the exp engines ~3us
    of slack before the PE blocks on a kt's attention weights (depth 8
    and larger rings both measured WORSE - this is a local optimum).
  - out-proj ec-halves allocate full-bank tiles from the scores ring
    (same tile size), staging copies split ACT/DVE.
"""

import numpy as np

import concourse.mybir as mybir
from concourse import bacc
from concourse.tile import TileContext
from concourse.masks import make_identity
from concourse.bass_utils import run_bass_kernel_spmd

B, S, D, H, HD = 2, 2048, 1024, 16, 64
GROUPS = 4
HG = H // GROUPS           # heads per core = 4
DV = HG * HD               # per-core qkv width = 256
P = 128
DC = D // P                # 8 contraction chunks
ST = S // P                # 16 k tiles
NQ = 512                   # q-chunk
QC = S // NQ               # 4 q-chunks
NCORES = 8

f32 = mybir.dt.float32
f32r = mybir.dt.float32r
bf16 = mybir.dt.bfloat16
i16 = mybir.dt.int16
EXP = mybir.ActivationFunctionType.Exp

SCH_A16 = 128.0 * 1.4426950408889634 / 8.0
SCH_B16 = 16250.4

# per-kt exp engine plan: halves (A,B); X=ACT exact, D=DVE Schraudolph.
# strict 16/16 XD/DX lockstep: any irregularity (XX kts, 17/15) measured
# far worse than the nominal imbalance it fixes.
HALF_PLAN = {}
for _kt in range(ST):
    HALF_PLAN[_kt] = "XD" if _kt % 2 == 0 else "DX"

_CACHE = {}
import os
ABLATE = frozenset(
    x for x in os.environ.get("V5_ABLATE", "").split(",") if x)


def _build(reps=1):
    nc = bacc.Bacc(None, target_bir_lowering=False, debug=False)

    xt_d = nc.dram_tensor("xt", [QC, P, DC, NQ], bf16, kind="ExternalInput")
    wqt_d = nc.dram_tensor("wqt", [P, DC, DV], bf16, kind="ExternalInput")
    wkt_d = nc.dram_tensor("wkt", [P, DC, DV], bf16, kind="ExternalInput")
    wvt_d = nc.dram_tensor("wvt", [P, DC, DV], bf16, kind="ExternalInput")
    wot_d = nc.dram_tensor("wot", [P, 2, D], bf16, kind="ExternalInput")
    out_d = nc.dram_tensor("out", [S, D], bf16, kind="ExternalOutput")

    from contextlib import ExitStack
    with TileContext(nc) as tc, ExitStack() as stack:
        if True:
            pp = stack.enter_context(tc.tile_pool(name="persist", bufs=1))
            ident = pp.tile([P, P], bf16)
            make_identity(nc, ident[:])

            qt_sb = pp.tile([P, 2, S], bf16)
            kt_sb = pp.tile([P, 2, S], bf16)
            vp_sb = pp.tile([P, ST, HG, HD + 1], bf16)
            ctn_sb = pp.tile([P, 2, S], bf16)
            wot_sb = pp.tile([P, 2, D], bf16)
            wq_sb = pp.tile([P, DC, DV], bf16)
            wk_sb = pp.tile([P, DC, DV], bf16)
            wv_sb = pp.tile([P, DC, DV], bf16)
            if "nonorm" in ABLATE:
                nc.any.memset(ctn_sb[:], 0.1)
            ones1 = pp.tile([P, 1], f32)
            nc.any.memset(ones1[:], 1.0)
            nc.vector.tensor_copy(
                vp_sb[:, :, :, HD:HD + 1],
                ones1.broadcast_to([P, ST, HG, 1]))

        if reps > 1:
            stack.enter_context(tc.For_i(0, reps, 1))
        if True:
            xtp = stack.enter_context(tc.tile_pool(name="xtp", bufs=2))
            ptp = stack.enter_context(tc.tile_pool(name="pt", bufs=20))
            ctt = stack.enter_context(tc.tile_pool(name="ctt", bufs=3))
            osb = stack.enter_context(tc.tile_pool(name="osb", bufs=4))
            wkp = stack.enter_context(tc.tile_pool(name="wkp", bufs=4))
            stp = stack.enter_context(
                tc.tile_pool(name="stp", bufs=5, space="PSUM"))
            ctp = stack.enter_context(
                tc.tile_pool(name="ctp", bufs=2, space="PSUM"))
            tpp = stack.enter_context(
                tc.tile_pool(name="tpp", bufs=1, space="PSUM"))
            # -------- interleaved initial DMAs ----------------------------
            xt_tiles = [None] * QC
            xt_tiles[0] = xtp.tile([P, DC, NQ], bf16, tag="xt", name="xt_sb")
            for dc in range(DC):
                nc.sync.dma_start(wq_sb[:, dc, :], wqt_d[:, dc, :])
                nc.sync.dma_start(xt_tiles[0][:, dc, :], xt_d[0, :, dc, :])
            for dc in range(DC):
                nc.sync.dma_start(wk_sb[:, dc, :], wkt_d[:, dc, :])
            for dc in range(DC):
                nc.sync.dma_start(wv_sb[:, dc, :], wvt_d[:, dc, :])
            xt_tiles[1] = xtp.tile([P, DC, NQ], bf16, tag="xt", name="xt_sb")
            nc.sync.dma_start(xt_tiles[1][:], xt_d[1])
            nc.sync.dma_start(wot_sb[:], wot_d[:])

            # -------- attention segment machinery -------------------------
            class Seg:
                """Heads (2t, 2t+1) x q[qlo:qlo+qw].  Scores stream per kt
                with a 2-deep deferred swapped-ctx queue.  ct tiles hold
                [q, qb-in-tile, head, hd|den] and are zero-initialized on
                DVE (interleaved accumulation groups cannot use start)."""

                def __init__(self, qlo, qw, t, alloc_now=False):
                    self.qlo, self.qw, self.t = qlo, qw, t
                    self.nqb = qw // P
                    self.ncts = (self.nqb + 1) // 2
                    self.cts = None
                    if alloc_now:
                        self.alloc_cts()
                    self.pending = []

                def alloc_cts(self):
                    self.cts = []
                    self.ct_virgin = []
                    for i in range(self.ncts):
                        ct = ctp.tile([P, 2, 2, HD + 1], f32,
                                      tag="ct", name="ct")
                        self.cts.append(ct)
                        self.ct_virgin.append(True)

                def _ctx(self, kt, pt_sb):
                    t = self.t
                    for qb in range(self.nqb):
                        ti, sub = divmod(qb, 2)
                        for h in (0, 1):
                            # first matmul into a fresh ct tile uses
                            # start=True: the bank-wide wipe zeroes all
                            # four interleaved accumulation regions.
                            st_f = self.ct_virgin[ti]
                            self.ct_virgin[ti] = False
                            nc.tensor.matmul(
                                self.cts[ti][:, sub, h, :],
                                pt_sb[:, h, qb * P:(qb + 1) * P],
                                vp_sb[:, kt, 2 * t + h, :],
                                start=st_f, stop=kt == ST - 1,
                                skip_group_check=True)

                def emit(self, kts, inject=None, depth=6):
                    qsl = slice(self.qlo, self.qlo + self.qw)
                    t, w = self.t, self.qw
                    for j, kt in enumerate(kts):
                        ksl = slice(kt * P, (kt + 1) * P)
                        stA = stp.tile([P, NQ], f32, tag="st", name="stA")
                        stB = stp.tile([P, NQ], f32, tag="st", name="stB")
                        pt_sb = ptp.tile([P, 2, NQ], bf16, tag="pt",
                                         name="pt_sb")
                        nc.tensor.matmul(
                            stA[:, 0:w], kt_sb[0:HD, t, ksl],
                            qt_sb[0:HD, t, qsl], tile_position=(0, 0))
                        nc.tensor.matmul(
                            stB[:, 0:w], kt_sb[HD:P, t, ksl],
                            qt_sb[HD:P, t, qsl], tile_position=(HD, 0))
                        for h, st_x in ((0, stA), (1, stB)):
                            typ = HALF_PLAN[kt][h]
                            if "noexp" in ABLATE:
                                nc.gpsimd.memset(pt_sb[:, h, 0:w], 0.5)
                                continue
                            if "alldve" in ABLATE:
                                typ = "D"
                            elif "allact" in ABLATE:
                                typ = "X"
                            if typ == "X":
                                nc.scalar.activation(
                                    pt_sb[:, h, 0:w], st_x[:, 0:w],
                                    EXP, scale=0.125)
                            else:
                                nc.vector.tensor_scalar(
                                    pt_sb.bitcast(i16)[:, h, 0:w],
                                    st_x[:, 0:w], SCH_A16, SCH_B16,
                                    mybir.AluOpType.mult,
                                    mybir.AluOpType.add)
                        if inject and j in inject:
                            for fn in inject[j]:
                                fn()
                        self.pending.append((kt, pt_sb))
                        if len(self.pending) > depth:
                            self._ctx(*self.pending.pop(0))
                    return self

                def flush(self):
                    for kt, pt_sb in self.pending:
                        self._ctx(kt, pt_sb)
                    self.pending = []
                    return self

            def norm_mul(seg):
                """DVE: reciprocal of denominators + normalize -> ctn_t."""
                ctn_t = ctt.tile([P, 4, 2, HD], bf16, tag="ctn_t",
                                 name="ctn_t")
                seg.ctn_t = ctn_t
                for i, ct in enumerate(seg.cts):
                    rcp = wkp.tile([P, 2, 2, 1], f32, tag="rcp", name="rcp")
                    with nc.allow_low_precision(
                            reason="softmax denom reciprocal"):
                        nc.vector.reciprocal(
                            rcp[:], ct[:, :, :, HD:HD + 1])
                    nc.vector.tensor_mul(
                        ctn_t[:, 2 * i:2 * i + 2, :, :],
                        ct[:, :, :, 0:HD],
                        rcp.broadcast_to([P, 2, 2, HD]))

            def norm_transpose(seg):
                tp = tpp.tile([P, 4, P], bf16, tag="tp", name="tp")
                seg.tp = tp
                for qb in range(seg.nqb):
                    nc.tensor.transpose(
                        tp[:, qb, :], seg.ctn_t[:, qb, :, :], ident[:])

            def norm_stage(seg):
                qsl = slice(seg.qlo, seg.qlo + seg.qw)
                nc.vector.tensor_copy(
                    ctn_sb[:, seg.t, qsl], seg.tp[:, 0:seg.nqb, :])

            def norm_all(seg):
                norm_mul(seg)
                norm_transpose(seg)
                norm_stage(seg)

            if "nonorm" in ABLATE:
                def norm_mul(seg):        # noqa: F811
                    ctn_t = ctt.tile([P, 4, 2, HD], bf16, tag="ctn_t",
                                     name="ctn_t")
                    seg.ctn_t = ctn_t
                    rcp = wkp.tile([P, 2, 2, 1], f32, tag="rcp",
                                   name="rcp")
                    for ct in seg.cts:
                        nc.vector.reciprocal(
                            rcp[:], ct[:, :, :, HD:HD + 1])

                def norm_transpose(seg):  # noqa: F811
                    pass

                def norm_stage(seg):      # noqa: F811
                    pass

            def outproj_sti(sti, split_dma=False):
                ssl = slice(sti * P, (sti + 1) * P)
                ob = osb.tile([P, D], bf16, tag="ob", name="ob")
                if "noout" in ABLATE:
                    nc.vector.tensor_copy(
                        ob[:], ctn_sb[:, 0, 0:D // 2].bitcast(bf16))
                    nc.sync.dma_start(out_d[ssl, :], ob[:])
                    return
                for ec in (0, 1):
                    esl = slice(ec * NQ, (ec + 1) * NQ)
                    op = stp.tile([P, NQ], f32, tag="st", name="op")
                    for dvt in (0, 1):
                        nc.tensor.matmul(
                            op[:],
                            ctn_sb[:, dvt, ssl],
                            wot_sb[:, dvt, esl],
                            start=dvt == 0, stop=dvt == 1)
                    if ec == 0:
                        nc.scalar.copy(ob[:, esl], op[:])
                    else:
                        nc.vector.tensor_copy(ob[:, esl], op[:])
                    if split_dma:
                        nc.sync.dma_start(out_d[ssl, esl], ob[:, esl])
                if not split_dma:
                    nc.sync.dma_start(out_d[ssl, :], ob[:])

            # -------- phase A: streamed loads + projections ---------------
            seg00 = Seg(0, NQ, 0, alloc_now=True)
            for sc in range(QC):
                ssl = slice(sc * NQ, (sc + 1) * NQ)
                xt_sb = xt_tiles[sc]

                def proj_qk(t):
                    for w_sb, dst in ((wq_sb, qt_sb), (wk_sb, kt_sb)):
                        ps = stp.tile([P, NQ], f32, tag="st", name="ps")
                        for dc in range(DC):
                            nc.tensor.matmul(
                                ps[:],
                                w_sb[:, dc, t * P:(t + 1) * P],
                                xt_sb[:, dc, :],
                                start=dc == 0, stop=dc == DC - 1)
                        nc.vector.tensor_copy(dst[:, t, ssl], ps[:])

                proj_qk(0)
                proj_qk(1)
                for si in range(4):
                    sti = sc * 4 + si
                    ps = stp.tile([P, NQ], f32, tag="st", name="ps")
                    for dc in range(DC):
                        nc.tensor.matmul(
                            ps[:, :DV],
                            xt_sb[:, dc, si * P:(si + 1) * P],
                            wv_sb[:, dc, :],
                            start=dc == 0, stop=dc == DC - 1)
                    for h in range(HG):
                        nc.scalar.copy(
                            vp_sb[:, sti, h, 0:HD],
                            ps[:, h * HD:(h + 1) * HD])
                seg00.emit(range(sc * 4, sc * 4 + 4))
                if sc + 2 < QC:
                    xt_tiles[sc + 2] = xtp.tile([P, DC, NQ], bf16,
                                                tag="xt", name="xt_sb")
                    nc.sync.dma_start(xt_tiles[sc + 2][:], xt_d[sc + 2])
            seg00.flush()

            # -------- phase B: pipelined attention + norm + out-proj ------
            if "nophaseb" in ABLATE:
                norm_all(seg00)
                for sti in range(ST):
                    outproj_sti(sti)
                plan = []
            else:
                plan = [
                    (0, NQ, 1, None),
                    (NQ, NQ, 0, 0),
                    (NQ, NQ, 1, None),
                    (2 * NQ, NQ, 0, 1),
                    (2 * NQ, NQ, 1, None),
                    (3 * NQ, NQ, 0, 2),
                    (3 * NQ, 256, 1, None),
                    (3 * NQ + 256, 256, 1, 3),
                ]
            prev = seg00
            for qlo, qw, t, op_qc in plan:
                seg = Seg(qlo, qw, t)
                inject = {
                    1: [lambda s=prev: norm_mul(s),
                        lambda s=seg: s.alloc_cts()],
                    2: [lambda s=prev: norm_transpose(s)],
                    3: [lambda s=prev: norm_stage(s)],
                }
                if op_qc is not None:
                    stis = range(op_qc * 4, op_qc * 4 + 4)
                    if op_qc == 3:
                        stis = (12, 13)
                    for jj, sti in zip((6, 8, 10, 12), stis):
                        inject[jj] = [lambda s=sti: outproj_sti(s)]
                seg.emit(range(ST), inject).flush()
                prev = seg
            # tail
            if "nophaseb" not in ABLATE:
                norm_all(prev)
                outproj_sti(14, split_dma=True)
                outproj_sti(15, split_dma=True)

    nc.compile()
    return nc


def _get_nc():
    if "nc" not in _CACHE:
        _CACHE["nc"] = _build()
    return _CACHE["nc"]


def _pack_inputs(x, Wq, Wk, Wv, Wo):
    import ml_dtypes
    bf = ml_dtypes.bfloat16
    x = np.asarray(x, np.float32)
    in_maps = []
    for c in range(NCORES):
        b, g = divmod(c, GROUPS)
        sl = slice(g * DV, (g + 1) * DV)
        xtb = np.ascontiguousarray(x[b].T)            # [D, S]
        xt = np.ascontiguousarray(
            xtb.reshape(DC, P, QC, NQ).transpose(2, 1, 0, 3)).astype(bf)
        wqt = np.ascontiguousarray(
            np.asarray(Wq, np.float32)[sl, :].T
            .reshape(DC, P, DV).transpose(1, 0, 2)).astype(bf)
        wkt = np.ascontiguousarray(
            np.asarray(Wk, np.float32)[sl, :].T
            .reshape(DC, P, DV).transpose(1, 0, 2)).astype(bf)
        wvt = np.ascontiguousarray(
            np.asarray(Wv, np.float32)[sl, :].T
            .reshape(DC, P, DV).transpose(1, 0, 2)).astype(bf)
        wot = np.ascontiguousarray(
            np.asarray(Wo, np.float32)[:, sl].T
            .reshape(2, P, D).transpose(1, 0, 2)).astype(bf)
        in_maps.append({"xt": xt, "wqt": wqt, "wkt": wkt,
                        "wvt": wvt, "wot": wot})
    return in_maps


def kernel(x, Wq, Wk, Wv, Wo, bo, _trace=False):
    bo = np.asarray(bo, np.float32)
    in_maps = _pack_inputs(x, Wq, Wk, Wv, Wo)
    res = run_bass_kernel_spmd(
        _get_nc(), in_maps, core_ids=list(range(NCORES)), trace=_trace)
    _CACHE["last_result"] = res
    parts = [np.asarray(res.results[c]["out"]).astype(np.float32)
             for c in range(NCORES)]
    out = np.empty((B, S, D), np.float32)
    for b in range(B):
        acc = np.sum(np.stack(parts[GROUPS * b:GROUPS * (b + 1)]),
                     axis=0, dtype=np.float64)
        out[b] = (acc + bo.astype(np.float64)).astype(np.float32)
    return out


# revision 7
# speedup vs baseline: 1.1033x; 1.1033x over previous
"""Multi-head attention (B=2, S=2048, D=1024, H=16) on 8 trn2 NeuronCores.

v5: swapped-ctx redesign.  Measured facts this build is shaped around
(microbench on this backend):
  - matmul wall time ~= 0.516ns x moving-cols (+~4ns), independent of
    dtype, contraction depth, and stationary reload (ldweights is free).
  - two 64-contraction-row matmuls at disjoint row quadrants run fully
    concurrently IF they target different PSUM banks.
  - matmul start=True zeroes the WHOLE psum bank; tiles are allocated
    bank-granular, so start=True is safe per-tile, but interleaved
    accumulation groups inside one tile use DVE pre-zero + start=False.
  - ACT activation ~757ns per [128,512] tile; DVE tensor_scalar
    (Schraudolph exp) ~331ns; DVE copy ~466ns.
Design:
  - scores as baseline: quadrant-paired 64-contraction matmuls -> [k,q].
  - ctx SWAPPED: stationary = 128x128 pt block, moving = [V|1] (65
    cols) -> ct[q, h, hd|den] in PSUM; 8x65-col matmuls per kt (301ns)
    instead of 2x512-col (530ns).
  - normalization via per-partition denominator: reciprocal with
    free-dim-broadcast input + one mul -> no PE broadcast matmuls.
  - ctn transposed back to [d, q] with one PE transpose per 128-q block
    (identity built on gpsimd), staged to SBUF f32r by one DVE copy.
  - exp split 16 ACT / 16 DVE-Schraudolph per 32 half-tiles, strict
    XD/DX alternation (no double-ACT kts); PSUM reads throttle both
    engines to ~725ns/tile so the pair runs in lockstep.
  - ALL matmul operands bf16 (qt/kt/ctn/wot included): the PE costs
    ~300ns per f32r<->bf16 dtype switch, which dominated earlier
    revisions (~2 switches/kt ~= 77us).
  - ctx deferral depth 6 + 20-deep pt ring give the exp engines ~3us
    of slack before the PE blocks on a kt's attention weights (depth 8
    and larger rings both measured WORSE - this is a local optimum).
  - out-proj ec-halves allocate full-bank tiles from the scores ring
    (same tile size), staging copies split ACT/DVE.
"""

import numpy as np

import concourse.mybir as mybir
from concourse import bacc
from concourse.tile import TileContext
from concourse.masks import make_identity
from concourse.bass_utils import run_bass_kernel_spmd

B, S, D, H, HD = 2, 2048, 1024, 16, 64
GROUPS = 4
HG = H // GROUPS           # heads per core = 4
DV = HG * HD               # per-core qkv width = 256
P = 128
DC = D // P                # 8 contraction chunks
ST = S // P                # 16 k tiles
NQ = 512                   # q-chunk
QC = S // NQ               # 4 q-chunks
NCORES = 8

f32 = mybir.dt.float32
f32r = mybir.dt.float32r
bf16 = mybir.dt.bfloat16
i16 = mybir.dt.int16
EXP = mybir.ActivationFunctionType.Exp

SCH_A16 = 128.0 * 1.4426950408889634 / 8.0
SCH_B16 = 16250.4

# per-kt exp engine plan: halves (A,B); X=ACT exact, D=DVE Schraudolph.
# strict 16/16 XD/DX lockstep: any irregularity (XX kts, 17/15) measured
# far worse than the nominal imbalance it fixes.
HALF_PLAN = {}
for _kt in range(ST):
    HALF_PLAN[_kt] = "XD" if _kt % 2 == 0 else "DX"

_CACHE = {}
import os
ABLATE = frozenset(
    x for x in os.environ.get("V5_ABLATE", "").split(",") if x)


def _build(reps=1):
    nc = bacc.Bacc(None, target_bir_lowering=False, debug=False)

    xt_d = nc.dram_tensor("xt", [QC, P, DC, NQ], bf16, kind="ExternalInput")
    wqt_d = nc.dram_tensor("wqt", [P, DC, DV], bf16, kind="ExternalInput")
    wkt_d = nc.dram_tensor("wkt", [P, DC, DV], bf16, kind="ExternalInput")
    wvt_d = nc.dram_tensor("wvt", [P, DC, DV], bf16, kind="ExternalInput")
    wot_d = nc.dram_tensor("wot", [P, 2, D], bf16, kind="ExternalInput")
    out_d = nc.dram_tensor("out", [S, D], bf16, kind="ExternalOutput")

    from contextlib import ExitStack
    with TileContext(nc) as tc, ExitStack() as stack:
        if True:
            pp = stack.enter_context(tc.tile_pool(name="persist", bufs=1))
            ident = pp.tile([P, P], bf16)
            make_identity(nc, ident[:])

            qt_sb = pp.tile([P, 2, S], bf16)
            kt_sb = pp.tile([P, 2, S], bf16)
            vp_sb = pp.tile([P, ST, HG, HD + 1], bf16)
            ctn_sb = pp.tile([P, 2, S], bf16)
            wot_sb = pp.tile([P, 2, D], bf16)
            wq_sb = pp.tile([P, DC, DV], bf16)
            wk_sb = pp.tile([P, DC, DV], bf16)
            wv_sb = pp.tile([P, DC, DV], bf16)
            if "nonorm" in ABLATE:
                nc.any.memset(ctn_sb[:], 0.1)
            ones1 = pp.tile([P, 1], f32)
            nc.any.memset(ones1[:], 1.0)
            nc.vector.tensor_copy(
                vp_sb[:, :, :, HD:HD + 1],
                ones1.broadcast_to([P, ST, HG, 1]))

        if reps > 1:
            stack.enter_context(tc.For_i(0, reps, 1))
        if True:
            xtp = stack.enter_context(tc.tile_pool(name="xtp", bufs=2))
            ptp = stack.enter_context(tc.tile_pool(name="pt", bufs=20))
            ctt = stack.enter_context(tc.tile_pool(name="ctt", bufs=3))
            osb = stack.enter_context(tc.tile_pool(name="osb", bufs=4))
            wkp = stack.enter_context(tc.tile_pool(name="wkp", bufs=4))
            stp = stack.enter_context(
                tc.tile_pool(name="stp", bufs=5, space="PSUM"))
            ctp = stack.enter_context(
                tc.tile_pool(name="ctp", bufs=2, space="PSUM"))
            tpp = stack.enter_context(
                tc.tile_pool(name="tpp", bufs=1, space="PSUM"))
            # -------- interleaved initial DMAs ----------------------------
            xt_tiles = [None] * QC
            xt_tiles[0] = xtp.tile([P, DC, NQ], bf16, tag="xt", name="xt_sb")
            for dc in range(DC):
                nc.sync.dma_start(wq_sb[:, dc, :], wqt_d[:, dc, :])
                nc.sync.dma_start(xt_tiles[0][:, dc, :], xt_d[0, :, dc, :])
            # wk/wv/wot ride the second HWDGE queue (Activation) so the
            # prologue weight stream doesn't serialize behind the wq+xt
            # pairs on SP; ACT's queue is free this early in phase A.
            for dc in range(DC):
                nc.scalar.dma_start(wk_sb[:, dc, :], wkt_d[:, dc, :])
            for dc in range(DC):
                nc.scalar.dma_start(wv_sb[:, dc, :], wvt_d[:, dc, :])
            xt_tiles[1] = xtp.tile([P, DC, NQ], bf16, tag="xt", name="xt_sb")
            nc.sync.dma_start(xt_tiles[1][:], xt_d[1])
            nc.scalar.dma_start(wot_sb[:], wot_d[:])

            # -------- attention segment machinery -------------------------
            class Seg:
                """Heads (2t, 2t+1) x q[qlo:qlo+qw].  Scores stream per kt
                with a 2-deep deferred swapped-ctx queue.  ct tiles hold
                [q, qb-in-tile, head, hd|den] and are zero-initialized on
                DVE (interleaved accumulation groups cannot use start)."""

                def __init__(self, qlo, qw, t, alloc_now=False):
                    self.qlo, self.qw, self.t = qlo, qw, t
                    self.nqb = qw // P
                    self.ncts = (self.nqb + 1) // 2
                    self.cts = None
                    if alloc_now:
                        self.alloc_cts()
                    self.pending = []

                def alloc_cts(self):
                    self.cts = []
                    self.ct_virgin = []
                    for i in range(self.ncts):
                        ct = ctp.tile([P, 2, 2, HD + 1], f32,
                                      tag="ct", name="ct")
                        self.cts.append(ct)
                        self.ct_virgin.append(True)

                def _ctx(self, kt, pt_sb):
                    t = self.t
                    for qb in range(self.nqb):
                        ti, sub = divmod(qb, 2)
                        for h in (0, 1):
                            # first matmul into a fresh ct tile uses
                            # start=True: the bank-wide wipe zeroes all
                            # four interleaved accumulation regions.
                            st_f = self.ct_virgin[ti]
                            self.ct_virgin[ti] = False
                            nc.tensor.matmul(
                                self.cts[ti][:, sub, h, :],
                                pt_sb[:, h, qb * P:(qb + 1) * P],
                                vp_sb[:, kt, 2 * t + h, :],
                                start=st_f, stop=kt == ST - 1,
                                skip_group_check=True)

                def emit(self, kts, inject=None, depth=6):
                    qsl = slice(self.qlo, self.qlo + self.qw)
                    t, w = self.t, self.qw
                    for j, kt in enumerate(kts):
                        ksl = slice(kt * P, (kt + 1) * P)
                        stA = stp.tile([P, NQ], f32, tag="st", name="stA")
                        stB = stp.tile([P, NQ], f32, tag="st", name="stB")
                        pt_sb = ptp.tile([P, 2, NQ], bf16, tag="pt",
                                         name="pt_sb")
                        nc.tensor.matmul(
                            stA[:, 0:w], kt_sb[0:HD, t, ksl],
                            qt_sb[0:HD, t, qsl], tile_position=(0, 0))
                        nc.tensor.matmul(
                            stB[:, 0:w], kt_sb[HD:P, t, ksl],
                            qt_sb[HD:P, t, qsl], tile_position=(HD, 0))
                        for h, st_x in ((0, stA), (1, stB)):
                            typ = HALF_PLAN[kt][h]
                            if "noexp" in ABLATE:
                                nc.gpsimd.memset(pt_sb[:, h, 0:w], 0.5)
                                continue
                            if "alldve" in ABLATE:
                                typ = "D"
                            elif "allact" in ABLATE:
                                typ = "X"
                            if typ == "X":
                                nc.scalar.activation(
                                    pt_sb[:, h, 0:w], st_x[:, 0:w],
                                    EXP, scale=0.125)
                            else:
                                nc.vector.tensor_scalar(
                                    pt_sb.bitcast(i16)[:, h, 0:w],
                                    st_x[:, 0:w], SCH_A16, SCH_B16,
                                    mybir.AluOpType.mult,
                                    mybir.AluOpType.add)
                        if inject and j in inject:
                            for fn in inject[j]:
                                fn()
                        self.pending.append((kt, pt_sb))
                        if len(self.pending) > depth:
                            self._ctx(*self.pending.pop(0))
                    return self

                def flush(self):
                    for kt, pt_sb in self.pending:
                        self._ctx(kt, pt_sb)
                    self.pending = []
                    return self

            def norm_mul(seg):
                """DVE: reciprocal of denominators + normalize -> ctn_t."""
                ctn_t = ctt.tile([P, 4, 2, HD], bf16, tag="ctn_t",
                                 name="ctn_t")
                seg.ctn_t = ctn_t
                for i, ct in enumerate(seg.cts):
                    rcp = wkp.tile([P, 2, 2, 1], f32, tag="rcp", name="rcp")
                    with nc.allow_low_precision(
                            reason="softmax denom reciprocal"):
                        nc.vector.reciprocal(
                            rcp[:], ct[:, :, :, HD:HD + 1])
                    nc.vector.tensor_mul(
                        ctn_t[:, 2 * i:2 * i + 2, :, :],
                        ct[:, :, :, 0:HD],
                        rcp.broadcast_to([P, 2, 2, HD]))

            def norm_transpose(seg):
                tp = tpp.tile([P, 4, P], bf16, tag="tp", name="tp")
                seg.tp = tp
                for qb in range(seg.nqb):
                    nc.tensor.transpose(
                        tp[:, qb, :], seg.ctn_t[:, qb, :, :], ident[:])

            def norm_stage(seg):
                qsl = slice(seg.qlo, seg.qlo + seg.qw)
                nc.vector.tensor_copy(
                    ctn_sb[:, seg.t, qsl], seg.tp[:, 0:seg.nqb, :])

            def norm_all(seg):
                norm_mul(seg)
                norm_transpose(seg)
                norm_stage(seg)

            if "nonorm" in ABLATE:
                def norm_mul(seg):        # noqa: F811
                    ctn_t = ctt.tile([P, 4, 2, HD], bf16, tag="ctn_t",
                                     name="ctn_t")
                    seg.ctn_t = ctn_t
                    rcp = wkp.tile([P, 2, 2, 1], f32, tag="rcp",
                                   name="rcp")
                    for ct in seg.cts:
                        nc.vector.reciprocal(
                            rcp[:], ct[:, :, :, HD:HD + 1])

                def norm_transpose(seg):  # noqa: F811
                    pass

                def norm_stage(seg):      # noqa: F811
                    pass

            def outproj_sti(sti, split_dma=False):
                ssl = slice(sti * P, (sti + 1) * P)
                ob = osb.tile([P, D], bf16, tag="ob", name="ob")
                if "noout" in ABLATE:
                    nc.vector.tensor_copy(
                        ob[:], ctn_sb[:, 0, 0:D // 2].bitcast(bf16))
                    nc.sync.dma_start(out_d[ssl, :], ob[:])
                    return
                for ec in (0, 1):
                    esl = slice(ec * NQ, (ec + 1) * NQ)
                    op = stp.tile([P, NQ], f32, tag="st", name="op")
                    for dvt in (0, 1):
                        nc.tensor.matmul(
                            op[:],
                            ctn_sb[:, dvt, ssl],
                            wot_sb[:, dvt, esl],
                            start=dvt == 0, stop=dvt == 1)
                    if ec == 0:
                        nc.scalar.copy(ob[:, esl], op[:])
                    else:
                        nc.vector.tensor_copy(ob[:, esl], op[:])
                    if split_dma:
                        nc.sync.dma_start(out_d[ssl, esl], ob[:, esl])
                if not split_dma:
                    nc.sync.dma_start(out_d[ssl, :], ob[:])

            # -------- phase A: streamed loads + projections ---------------
            seg00 = Seg(0, NQ, 0, alloc_now=True)
            for sc in range(QC):
                ssl = slice(sc * NQ, (sc + 1) * NQ)
                xt_sb = xt_tiles[sc]

                def proj_qk(t):
                    for w_sb, dst in ((wq_sb, qt_sb), (wk_sb, kt_sb)):
                        ps = stp.tile([P, NQ], f32, tag="st", name="ps")
                        for dc in range(DC):
                            nc.tensor.matmul(
                                ps[:],
                                w_sb[:, dc, t * P:(t + 1) * P],
                                xt_sb[:, dc, :],
                                start=dc == 0, stop=dc == DC - 1)
                        nc.vector.tensor_copy(dst[:, t, ssl], ps[:])

                proj_qk(0)
                proj_qk(1)
                for si in range(4):
                    sti = sc * 4 + si
                    ps = stp.tile([P, NQ], f32, tag="st", name="ps")
                    for dc in range(DC):
                        nc.tensor.matmul(
                            ps[:, :DV],
                            xt_sb[:, dc, si * P:(si + 1) * P],
                            wv_sb[:, dc, :],
                            start=dc == 0, stop=dc == DC - 1)
                    for h in range(HG):
                        nc.scalar.copy(
                            vp_sb[:, sti, h, 0:HD],
                            ps[:, h * HD:(h + 1) * HD])
                seg00.emit(range(sc * 4, sc * 4 + 4))
                if sc + 2 < QC:
                    xt_tiles[sc + 2] = xtp.tile([P, DC, NQ], bf16,
                                                tag="xt", name="xt_sb")
                    nc.sync.dma_start(xt_tiles[sc + 2][:], xt_d[sc + 2])
            seg00.flush()

            # -------- phase B: pipelined attention + norm + out-proj ------
            if "nophaseb" in ABLATE:
                norm_all(seg00)
                for sti in range(ST):
                    outproj_sti(sti)
                plan = []
            else:
                plan = [
                    (0, NQ, 1, None),
                    (NQ, NQ, 0, 0),
                    (NQ, NQ, 1, None),
                    (2 * NQ, NQ, 0, 1),
                    (2 * NQ, NQ, 1, None),
                    (3 * NQ, NQ, 0, 2),
                    (3 * NQ, 256, 1, None),
                    (3 * NQ + 256, 256, 1, 3),
                ]
            prev = seg00
            for qlo, qw, t, op_qc in plan:
                seg = Seg(qlo, qw, t)
                inject = {
                    1: [lambda s=prev: norm_mul(s),
                        lambda s=seg: s.alloc_cts()],
                    2: [lambda s=prev: norm_transpose(s)],
                    3: [lambda s=prev: norm_stage(s)],
                }
                if op_qc is not None:
                    stis = range(op_qc * 4, op_qc * 4 + 4)
                    if op_qc == 3:
                        stis = (12, 13)
                    for jj, sti in zip((6, 8, 10, 12), stis):
                        inject[jj] = [lambda s=sti: outproj_sti(s)]
                seg.emit(range(ST), inject).flush()
                prev = seg
            # tail
            if "nophaseb" not in ABLATE:
                norm_all(prev)
                outproj_sti(14, split_dma=True)
                outproj_sti(15, split_dma=True)

    nc.compile()
    return nc


def _get_nc():
    if "nc" not in _CACHE:
        _CACHE["nc"] = _build()
    return _CACHE["nc"]


def _pack_inputs(x, Wq, Wk, Wv, Wo):
    import ml_dtypes
    bf = ml_dtypes.bfloat16
    x = np.asarray(x, np.float32)
    in_maps = []
    for c in range(NCORES):
        b, g = divmod(c, GROUPS)
        sl = slice(g * DV, (g + 1) * DV)
        xtb = np.ascontiguousarray(x[b].T)            # [D, S]
        xt = np.ascontiguousarray(
            xtb.reshape(DC, P, QC, NQ).transpose(2, 1, 0, 3)).astype(bf)
        wqt = np.ascontiguousarray(
            np.asarray(Wq, np.float32)[sl, :].T
            .reshape(DC, P, DV).transpose(1, 0, 2)).astype(bf)
        wkt = np.ascontiguousarray(
            np.asarray(Wk, np.float32)[sl, :].T
            .reshape(DC, P, DV).transpose(1, 0, 2)).astype(bf)
        wvt = np.ascontiguousarray(
            np.asarray(Wv, np.float32)[sl, :].T
            .reshape(DC, P, DV).transpose(1, 0, 2)).astype(bf)
        wot = np.ascontiguousarray(
            np.asarray(Wo, np.float32)[:, sl].T
            .reshape(2, P, D).transpose(1, 0, 2)).astype(bf)
        in_maps.append({"xt": xt, "wqt": wqt, "wkt": wkt,
                        "wvt": wvt, "wot": wot})
    return in_maps


def kernel(x, Wq, Wk, Wv, Wo, bo, _trace=False):
    bo = np.asarray(bo, np.float32)
    in_maps = _pack_inputs(x, Wq, Wk, Wv, Wo)
    res = run_bass_kernel_spmd(
        _get_nc(), in_maps, core_ids=list(range(NCORES)), trace=_trace)
    _CACHE["last_result"] = res
    parts = [np.asarray(res.results[c]["out"]).astype(np.float32)
             for c in range(NCORES)]
    out = np.empty((B, S, D), np.float32)
    for b in range(B):
        acc = np.sum(np.stack(parts[GROUPS * b:GROUPS * (b + 1)]),
                     axis=0, dtype=np.float64)
        out[b] = (acc + bo.astype(np.float64)).astype(np.float32)
    return out
